# revision 5
# baseline (speedup 1.0000x reference)
"""Trainium2 Bass kernel for DigitConvolutionalModel forward pass.

Model: x[B,784] -> 3x3 valid conv (single channel) -> flatten[676]
       -> relu(.@W1+b1) -> relu(.@W2+b2) -> .@W3+b3 -> [B,10]

Strategy:
  - Pure data parallel: batch 32768 sharded 8 ways (4096 rows/core);
    weights replicated.
  - The conv is linear, so it folds into the first Linear layer:
        conv(x).flat @ W1 == x @ (C @ W1)
    where C[784,676] is the conv-as-matrix. The host computes
    W1p = C @ W1 directly and pre-transposes x to pixel-major
    [784, 4096] per core, so the device DMAs matmul-ready [pix, batch]
    tiles and the PE spends zero cycles transposing inputs.
  - All matmul operands are bf16 (full-rate PE path); PSUM accumulation
    stays fp32, biases applied in fp32 on ScalarE during PSUM eviction.
    End-to-end bf16 error ~5e-3, well under the 2e-2 gate.
  - Pixel dim is chunked 7x112 so each batch-tile group loads with ONE
    large DMA; all weights+biases are packed into two [128, *] blobs
    loaded with one DMA each (per-DMA descriptor generation dominated
    the v3 startup). Input group DMAs alternate between the two HWDGE
    queues (sync + scalar engines); everything is pre-issued.
  - Output is stored TRANSPOSED [10, 4096] (host transposes back):
    the ps3 eviction lands batch-in-free-dim, so stores are 10
    contiguous 2KB descriptors instead of 512 40-byte ones, and no
    PE transpose / DVE cast is needed on the output path.
  - Last two tiles split into 256-col halves to pipeline the drain.
"""

import sys

for _p in (
    "/opt/trn_rl_repo",
    "/root/.axon_site",
    "/root/.axon_site/_ro/trn_rl_repo",
    "/root/.axon_site/_ro/pypackages",
):
    if _p not in sys.path:
        sys.path.append(_p)

from contextlib import ExitStack

import numpy as np
import ml_dtypes

import concourse.bass as bass
import concourse.tile as tile
from concourse import mybir
from concourse.bass_utils import run_bass_kernel_spmd
from concourse.masks import make_identity

F32 = mybir.dt.float32
BF16 = mybir.dt.bfloat16
AFT = mybir.ActivationFunctionType
NP_BF16 = ml_dtypes.bfloat16

B_FULL = 32768
N_CORES = 8
B_CORE = B_FULL // N_CORES  # 4096
IMG = 28
OHW = 26
FLAT = OHW * OHW  # 676
NPIX = IMG * IMG  # 784
HID = 300
NCLS = 10

BT = 512  # batch tile (matmul moving free dim; PSUM bank = 512 fp32)
NBT = B_CORE // BT  # 8

PW = 112  # pixel chunk: 784 = 7*112, uniform -> single-DMA group loads
NPC = NPIX // PW  # 7
H_CH = [(s, min(128, HID - s)) for s in range(0, HID, 128)]  # 3 chunks

# weight blob column offsets (bf16 blob [128, WCOLS])
W1P_OFF = 0  # 7 chunks of [112, 300]
W2_OFF = NPC * HID  # 2100; 3 chunks of [hp, 300]
W3_OFF = W2_OFF + 3 * HID  # 3000; 3 chunks of [hp, 10]
WCOLS = W3_OFF + 3 * NCLS  # 3030

# batch-tile groups, each loaded by one DMA: [1, 2, 2, 2, 1] tiles
GROUPS = [(0, 1), (1, 2), (3, 2), (5, 2), (7, 1)]


def _legalize_single_wait(nc):
    """This walrus build accepts only one sync-wait per instruction; move
    extra waits onto NoOps inserted just before, on the same engine."""
    n = 0
    for fn in nc.m.functions:
        for bb in fn.blocks:
            new_insts = []
            for inst in bb.instructions:
                si = inst.sync_info
                if si is not None and si.on_wait and len(si.on_wait) > 1:
                    waits = list(si.on_wait)
                    for w in waits[:-1]:
                        nop = mybir.InstNoOp(
                            name=f"{inst.name}-w{n}",
                            sync_info=mybir.SyncInfo(on_wait=[w], on_update=[]),
                            bass_nofuse=True,
                            engine=inst.engine,
                        )
                        n += 1
                        nc.register_instruction(nop, overwrite=True)
                        new_insts.append(nop)
                    inst.sync_info = mybir.SyncInfo(
                        on_wait=[waits[-1]], on_update=list(si.on_update)
                    )
                new_insts.append(inst)
            bb.instructions = new_insts
    return n


def _emit(ctx: ExitStack, tc: tile.TileContext, xt_d, wb, bb, out):
    nc = tc.nc

    const = ctx.enter_context(tc.tile_pool(name="const", bufs=1))
    psmm = ctx.enter_context(tc.tile_pool(name="psmm", bufs=6, space="PSUM"))
    hp_ = ctx.enter_context(tc.tile_pool(name="hp", bufs=2))
    obp = ctx.enter_context(tc.tile_pool(name="obp", bufs=8))

    ident = const.tile([128, 128], F32, name="ident")
    make_identity(nc, ident)
    identb = const.tile([128, 128], BF16, name="identb")
    nc.vector.tensor_copy(identb[:, :], ident[:, :])

    # --- all DMAs pre-issued; weights first on the scalar HWDGE queue ---
    bbt = const.tile([128, 7], F32, name="bbt")
    nc.scalar.dma_start(bbt[:, :], bb[:, :])
    wbt = const.tile([128, WCOLS], BF16, name="wbt")
    nc.scalar.dma_start(wbt[:, :], wb[:, :])
    w1ps = [wbt[0:PW, W1P_OFF + pc * HID : W1P_OFF + (pc + 1) * HID] for pc in range(NPC)]
    w2s = [wbt[0:hp, W2_OFF + hc * HID : W2_OFF + hc * HID + HID] for hc, (h0, hp) in enumerate(H_CH)]
    w3s = [wbt[0:hp, W3_OFF + hc * NCLS : W3_OFF + (hc + 1) * NCLS] for hc, (h0, hp) in enumerate(H_CH)]
    b1s = [bbt[0:hp, hc : hc + 1] for hc, (h0, hp) in enumerate(H_CH)]
    b2s = [bbt[0:hp, 3 + hc : 4 + hc] for hc, (h0, hp) in enumerate(H_CH)]
    b3s = bbt[0:NCLS, 6:7]

    # x.T view [PW, NPC, B] so one DMA loads all 7 pixel chunks of a group
    xt_v = xt_d.rearrange("(c p) b -> p c b", c=NPC)
    xg = []
    for gi, (t0, nt) in enumerate(GROUPS):
        g = const.tile([PW, NPC, nt * BT], BF16, name=f"xg{gi}")
        eng = nc.sync if gi % 2 == 0 else nc.scalar
        eng.dma_start(g[:, :, :], xt_v[:, :, t0 * BT : (t0 + nt) * BT])
        xg.append(g)

    # Dense PE warmup burst: keeps the HAM clock gate ramping while the
    # first input group + weight blob DMA in.
    warm = psmm.tile([128, BT], F32, name="warm", tag="psf")
    for _ in range(55):
        nc.tensor.matmul(
            warm[0:128, 0:128], identb[:, :], identb[:, :],
            start=True, stop=True,
        )

    # tile index -> (group tile view, col offset of tile in group)
    tview = {}
    for gi, (t0, nt) in enumerate(GROUPS):
        for k in range(nt):
            tview[t0 + k] = (xg[gi], k * BT)

    def compute(t, off, n):
        """fc1->fc2->fc3->store for batch columns [off, off+n) of tile t."""
        g, goff = tview[t]
        c0 = goff + off
        # fc1: relu(x @ W1p + b1), hidden-major [300, n]; k-outer so the
        # first matmuls only need the first pixel chunk of the group DMA
        ps1 = [psmm.tile([128, BT], F32, name=f"ps1_{hc}", tag="psf") for hc in range(3)]
        for pc in range(NPC):
            for hc, (h0, hp) in enumerate(H_CH):
                nc.tensor.matmul(
                    ps1[hc][0:hp, 0:n],
                    w1ps[pc][0:PW, h0 : h0 + hp],
                    g[0:PW, pc, c0 : c0 + n],
                    start=(pc == 0),
                    stop=(pc == NPC - 1),
                )
        h1 = []
        for hc, (h0, hp) in enumerate(H_CH):
            h = hp_.tile([hp, BT], BF16, name=f"h1_{hc}", tag=f"h1_{hc}")
            nc.scalar.activation(
                h[:, 0:n], ps1[hc][0:hp, 0:n], AFT.Relu, bias=b1s[hc]
            )
            h1.append(h)

        # fc2: relu(h1 @ W2 + b2) — k-outer so all m-groups unblock on h1[0]
        ps2 = [psmm.tile([128, BT], F32, name=f"ps2_{hc2}", tag="psf") for hc2 in range(3)]
        for hc, (h0, hp) in enumerate(H_CH):
            for hc2, (g0, gp) in enumerate(H_CH):
                nc.tensor.matmul(
                    ps2[hc2][0:gp, 0:n],
                    w2s[hc][0:hp, g0 : g0 + gp],
                    h1[hc][0:hp, 0:n],
                    start=(hc == 0),
                    stop=(hc == len(H_CH) - 1),
                )
        h2 = []
        for hc2, (g0, gp) in enumerate(H_CH):
            h = hp_.tile([gp, BT], BF16, name=f"h2_{hc2}", tag=f"h2_{hc2}")
            nc.scalar.activation(
                h[:, 0:n], ps2[hc2][0:gp, 0:n], AFT.Relu, bias=b2s[hc2]
            )
            h2.append(h)

        # fc3: h2 @ W3 + b3 -> [10, n], stored transposed (host fixes up)
        ps = psmm.tile([128, BT], F32, name="ps3", tag="psf")
        for hc, (h0, hp) in enumerate(H_CH):
            nc.tensor.matmul(
                ps[0:NCLS, 0:n],
                w3s[hc][0:hp, 0:NCLS],
                h2[hc][0:hp, 0:n],
                start=(hc == 0),
                stop=(hc == len(H_CH) - 1),
            )
        os_ = obp.tile([NCLS, BT], F32, name="os", tag="os")
        nc.scalar.activation(
            os_[:, 0:n], ps[0:NCLS, 0:n], AFT.Identity, bias=b3s
        )
        r0 = t * BT
        nc.sync.dma_start(
            out[:, r0 + off : r0 + off + n], os_[:, 0:n]
        )

    for t in range(NBT):
        if t >= NBT - 2:
            # split the last two tiles to pipeline the serial drain chain
            compute(t, 0, 256)
            compute(t, 256, 256)
        else:
            compute(t, 0, BT)


def _build_c(conv_w: np.ndarray) -> np.ndarray:
    """C[p, q] with conv(x).flat = x @ C. Pure scatter of conv_w."""
    c = np.zeros((NPIX, FLAT), np.float32)
    oi = np.arange(OHW)
    oj = np.arange(OHW)
    q = (oi[:, None] * OHW + oj[None, :]).ravel()
    for dy in range(3):
        for dx in range(3):
            p = ((oi[:, None] + dy) * IMG + (oj[None, :] + dx)).ravel()
            c[p, q] = conv_w[dy, dx]
    return c


_NC_CACHE: list = []


def _get_nc():
    if _NC_CACHE:
        return _NC_CACHE[0]
    nc = bass.Bass("TRN2", target_bir_lowering=False, debug=False)
    xt_d = nc.dram_tensor("xt", [NPIX, B_CORE], BF16, kind="ExternalInput").ap()
    wb = nc.dram_tensor("wb", [128, WCOLS], BF16, kind="ExternalInput").ap()
    bb = nc.dram_tensor("bb", [128, 7], F32, kind="ExternalInput").ap()
    out = nc.dram_tensor("out", [NCLS, B_CORE], F32, kind="ExternalOutput").ap()
    with tile.TileContext(nc) as tc:
        with ExitStack() as ctx:
            _emit(ctx, tc, xt_d, wb, bb, out)
    _legalize_single_wait(nc)
    _NC_CACHE.append(nc)
    return nc


def _in_maps(inputs: dict) -> list:
    x = np.asarray(inputs["x"], dtype=np.float32)
    assert x.shape == (B_FULL, NPIX), x.shape
    c = _build_c(np.asarray(inputs["conv_w"], dtype=np.float32))
    w1p = (c @ np.asarray(inputs["W1"], np.float32)).astype(NP_BF16)
    w2 = np.asarray(inputs["W2"], np.float32).astype(NP_BF16)
    w3 = np.asarray(inputs["W3"], np.float32).astype(NP_BF16)
    wb = np.zeros((128, WCOLS), NP_BF16)
    for pc in range(NPC):
        wb[0:PW, W1P_OFF + pc * HID : W1P_OFF + (pc + 1) * HID] = w1p[
            pc * PW : (pc + 1) * PW
        ]
    for hc, (h0, hp) in enumerate(H_CH):
        wb[0:hp, W2_OFF + hc * HID : W2_OFF + hc * HID + HID] = w2[h0 : h0 + hp]
        wb[0:hp, W3_OFF + hc * NCLS : W3_OFF + (hc + 1) * NCLS] = w3[h0 : h0 + hp]
    bbl = np.zeros((128, 7), np.float32)
    b1 = np.asarray(inputs["b1"], np.float32)
    b2 = np.asarray(inputs["b2"], np.float32)
    b3 = np.asarray(inputs["b3"], np.float32)
    for hc, (h0, hp) in enumerate(H_CH):
        bbl[0:hp, hc] = b1[h0 : h0 + hp]
        bbl[0:hp, 3 + hc] = b2[h0 : h0 + hp]
    bbl[0:NCLS, 6] = b3
    xb = x.astype(NP_BF16)
    common = {"wb": wb, "bb": bbl}
    return [
        {
            "xt": np.ascontiguousarray(xb[c_ * B_CORE : (c_ + 1) * B_CORE].T),
            **common,
        }
        for c_ in range(N_CORES)
    ]


def _gather(res) -> np.ndarray:
    return np.concatenate(
        [np.ascontiguousarray(res.results[c]["out"].T) for c in range(N_CORES)],
        axis=0,
    )


def kernel(**inputs) -> np.ndarray:
    nc = _get_nc()
    res = run_bass_kernel_spmd(nc, _in_maps(inputs), list(range(N_CORES)))
    return _gather(res)


if __name__ == "__main__":
    rng = np.random.default_rng(0)
    ins = {
        "x": rng.standard_normal((B_FULL, NPIX), dtype=np.float32),
        "conv_w": rng.standard_normal((3, 3), dtype=np.float32) * 0.1,
        "W1": rng.standard_normal((FLAT, HID), dtype=np.float32) * 0.04,
        "b1": np.zeros(HID, np.float32),
        "W2": rng.standard_normal((HID, HID), dtype=np.float32) * 0.06,
        "b2": np.zeros(HID, np.float32),
        "W3": rng.standard_normal((HID, NCLS), dtype=np.float32) * 0.06,
        "b3": np.zeros(NCLS, np.float32),
    }
    y = kernel(**ins)
    # numpy reference with explicit conv
    from numpy.lib.stride_tricks import sliding_window_view

    img = ins["x"].reshape(-1, IMG, IMG)
    win = sliding_window_view(img, (3, 3), axis=(1, 2))
    conv = np.einsum("bijkl,kl->bij", win, ins["conv_w"]).reshape(-1, FLAT)
    h = np.maximum(conv @ ins["W1"] + ins["b1"], 0)
    h = np.maximum(h @ ins["W2"] + ins["b2"], 0)
    ref = h @ ins["W3"] + ins["b3"]
    err = np.abs(y - ref).max() / (np.abs(ref).max() + 1e-9)
    print("max rel err vs numpy:", err)


# revision 8
# speedup vs baseline: 1.2328x; 1.2328x over previous
"""Trainium2 Bass kernel for DigitConvolutionalModel forward pass.

Model: x[B,784] -> 3x3 valid conv (single channel) -> flatten[676]
       -> relu(.@W1+b1) -> relu(.@W2+b2) -> .@W3+b3 -> [B,10]

Strategy:
  - Pure data parallel: batch 32768 sharded 8 ways (4096 rows/core);
    weights replicated.
  - The conv is linear, so it folds into the first Linear layer:
        conv(x).flat @ W1 == x @ (C @ W1)
    where C[784,676] is the conv-as-matrix. The host computes
    W1p = C @ W1 directly and pre-transposes x to pixel-major
    [784, 4096] per core, so the device DMAs matmul-ready [pix, batch]
    tiles and the PE spends zero cycles transposing inputs.
  - All matmul operands are bf16 (full-rate PE path); PSUM accumulation
    stays fp32, biases applied in fp32 on ScalarE during PSUM eviction.
    End-to-end bf16 error ~5e-3, well under the 2e-2 gate.
  - Pixel dim is chunked 7x112 so each batch-tile group loads with ONE
    large DMA; all weights+biases are packed into two [128, *] blobs
    loaded with one DMA each (per-DMA descriptor generation dominated
    the v3 startup). Input group DMAs alternate between the two HWDGE
    queues (sync + scalar engines); everything is pre-issued.
  - Output is stored TRANSPOSED [10, 4096] (host transposes back):
    the ps3 eviction lands batch-in-free-dim, so stores are 10
    contiguous 2KB descriptors instead of 512 40-byte ones, and no
    PE transpose / DVE cast is needed on the output path.
  - Last two tiles split into 256-col halves to pipeline the drain.
"""

import sys

for _p in (
    "/opt/trn_rl_repo",
    "/root/.axon_site",
    "/root/.axon_site/_ro/trn_rl_repo",
    "/root/.axon_site/_ro/pypackages",
):
    if _p not in sys.path:
        sys.path.append(_p)

from contextlib import ExitStack

import numpy as np
import ml_dtypes

import concourse.bass as bass
import concourse.tile as tile
from concourse import mybir
from concourse.bass_utils import run_bass_kernel_spmd
from concourse.masks import make_identity

F32 = mybir.dt.float32
BF16 = mybir.dt.bfloat16
AFT = mybir.ActivationFunctionType
NP_BF16 = ml_dtypes.bfloat16

B_FULL = 32768
N_CORES = 8
B_CORE = B_FULL // N_CORES  # 4096
IMG = 28
OHW = 26
FLAT = OHW * OHW  # 676
NPIX = IMG * IMG  # 784
HID = 300
NCLS = 10

BT = 512  # batch tile (matmul moving free dim; PSUM bank = 512 fp32)
NBT = B_CORE // BT  # 8

PW = 112  # pixel chunk: 784 = 7*112, uniform -> single-DMA group loads
NPC = NPIX // PW  # 7
H_CH = [(s, min(128, HID - s)) for s in range(0, HID, 128)]  # 3 chunks

# weight blob column offsets (bf16 blob [128, WCOLS])
W1P_OFF = 0  # 7 chunks of [112, 300]
W2_OFF = NPC * HID  # 2100; 3 chunks of [hp, 300]
W3_OFF = W2_OFF + 3 * HID  # 3000; 3 chunks of [hp, 10]
WCOLS = W3_OFF + 3 * NCLS  # 3030

# batch-tile groups, each loaded by one DMA: [1, 2, 2, 2, 1] tiles
GROUPS = [(0, 1), (1, 2), (3, 2), (5, 2), (7, 1)]


def _legalize_single_wait(nc):
    """This walrus build accepts only one sync-wait per instruction; move
    extra waits onto NoOps inserted just before, on the same engine."""
    n = 0
    for fn in nc.m.functions:
        for bb in fn.blocks:
            new_insts = []
            for inst in bb.instructions:
                si = inst.sync_info
                if si is not None and si.on_wait and len(si.on_wait) > 1:
                    waits = list(si.on_wait)
                    for w in waits[:-1]:
                        nop = mybir.InstNoOp(
                            name=f"{inst.name}-w{n}",
                            sync_info=mybir.SyncInfo(on_wait=[w], on_update=[]),
                            bass_nofuse=True,
                            engine=inst.engine,
                        )
                        n += 1
                        nc.register_instruction(nop, overwrite=True)
                        new_insts.append(nop)
                    inst.sync_info = mybir.SyncInfo(
                        on_wait=[waits[-1]], on_update=list(si.on_update)
                    )
                new_insts.append(inst)
            bb.instructions = new_insts
    return n


def _emit(ctx: ExitStack, tc: tile.TileContext, xt_d, wb, bb, out):
    nc = tc.nc

    const = ctx.enter_context(tc.tile_pool(name="const", bufs=1))
    psmm = ctx.enter_context(tc.tile_pool(name="psmm", bufs=8, space="PSUM"))
    hp_ = ctx.enter_context(tc.tile_pool(name="hp", bufs=2))
    obp = ctx.enter_context(tc.tile_pool(name="obp", bufs=8))

    ident = const.tile([128, 128], F32, name="ident")
    make_identity(nc, ident)
    identb = const.tile([128, 128], BF16, name="identb")
    nc.vector.tensor_copy(identb[:, :], ident[:, :])

    # --- all DMAs pre-issued; weights first on the scalar HWDGE queue ---
    bbt = const.tile([128, 7], F32, name="bbt")
    nc.scalar.dma_start(bbt[:, :], bb[:, :])
    wbt = const.tile([128, WCOLS], BF16, name="wbt")
    nc.scalar.dma_start(wbt[:, :], wb[:, :])
    w1ps = [wbt[0:PW, W1P_OFF + pc * HID : W1P_OFF + (pc + 1) * HID] for pc in range(NPC)]
    w2s = [wbt[0:hp, W2_OFF + hc * HID : W2_OFF + hc * HID + HID] for hc, (h0, hp) in enumerate(H_CH)]
    w3s = [wbt[0:hp, W3_OFF + hc * NCLS : W3_OFF + (hc + 1) * NCLS] for hc, (h0, hp) in enumerate(H_CH)]
    b1s = [bbt[0:hp, hc : hc + 1] for hc, (h0, hp) in enumerate(H_CH)]
    b2s = [bbt[0:hp, 3 + hc : 4 + hc] for hc, (h0, hp) in enumerate(H_CH)]
    b3s = bbt[0:NCLS, 6:7]

    # x.T view [PW, NPC, B] so one DMA loads all 7 pixel chunks of a group
    xt_v = xt_d.rearrange("(c p) b -> p c b", c=NPC)
    xg = []
    for gi, (t0, nt) in enumerate(GROUPS):
        g = const.tile([PW, NPC, nt * BT], BF16, name=f"xg{gi}")
        eng = nc.sync if gi % 2 == 0 else nc.scalar
        eng.dma_start(g[:, :, :], xt_v[:, :, t0 * BT : (t0 + nt) * BT])
        xg.append(g)

    # Dense PE warmup burst: keeps the HAM clock gate ramping while the
    # first input group + weight blob DMA in.
    warm = psmm.tile([128, BT], F32, name="warm", tag="psf")
    for _ in range(55):
        nc.tensor.matmul(
            warm[0:128, 0:128], identb[:, :], identb[:, :],
            start=True, stop=True,
        )

    # tile index -> (group tile view, col offset of tile in group)
    tview = {}
    for gi, (t0, nt) in enumerate(GROUPS):
        for k in range(nt):
            tview[t0 + k] = (xg[gi], k * BT)

    # --- software-pipelined main loop: iteration I_t issues
    #     fc1(t+1) | fc2(t) | fc3(t-1)
    # so the PE never stalls at an fc boundary waiting for a ScalarE
    # PSUM eviction — the evictions of stage s(t) overlap the ~4.5us of
    # fc1(t+1) matmuls. PSUM live set: 3 (ps1) + 3 (ps2) + 1 (ps3) = 7
    # of 8 banks. All matmul runs are same-PSUM-bank chains (hc-outer /
    # m-outer): consecutive bank-switching accumulation was measured
    # ~1.2us/tile slower.
    h1v, h2v = {}, {}

    def fc1(t):
        g, goff = tview[t]
        h1 = []
        for hc, (h0, hp) in enumerate(H_CH):
            ps = psmm.tile([128, BT], F32, name=f"ps1_{hc}", tag="psf")
            for pc in range(NPC):
                nc.tensor.matmul(
                    ps[0:hp, 0:BT],
                    w1ps[pc][0:PW, h0 : h0 + hp],
                    g[0:PW, pc, goff : goff + BT],
                    start=(pc == 0),
                    stop=(pc == NPC - 1),
                )
            h = hp_.tile([hp, BT], BF16, name=f"h1_{hc}", tag=f"h1_{hc}")
            nc.scalar.activation(h[:, :], ps[0:hp, 0:BT], AFT.Relu, bias=b1s[hc])
            h1.append(h)
        h1v[t] = h1

    def fc2(t):
        h1 = h1v.pop(t)
        h2 = []
        for hc2, (g0, gp) in enumerate(H_CH):
            ps = psmm.tile([128, BT], F32, name=f"ps2_{hc2}", tag="psf")
            for hc, (h0, hp) in enumerate(H_CH):
                nc.tensor.matmul(
                    ps[0:gp, 0:BT],
                    w2s[hc][0:hp, g0 : g0 + gp],
                    h1[hc][0:hp, 0:BT],
                    start=(hc == 0),
                    stop=(hc == len(H_CH) - 1),
                )
            h = hp_.tile([gp, BT], BF16, name=f"h2_{hc2}", tag=f"h2_{hc2}")
            nc.scalar.activation(h[:, :], ps[0:gp, 0:BT], AFT.Relu, bias=b2s[hc2])
            h2.append(h)
        h2v[t] = h2

    def fc3(t):
        h2 = h2v.pop(t)
        ps = psmm.tile([128, BT], F32, name="ps3", tag="psf")
        for hc, (h0, hp) in enumerate(H_CH):
            nc.tensor.matmul(
                ps[0:NCLS, 0:BT],
                w3s[hc][0:hp, 0:NCLS],
                h2[hc][0:hp, 0:BT],
                start=(hc == 0),
                stop=(hc == len(H_CH) - 1),
            )
        os_ = obp.tile([NCLS, BT], F32, name="os", tag="os")
        nc.scalar.activation(os_[:, :], ps[0:NCLS, 0:BT], AFT.Identity, bias=b3s)
        r0 = t * BT
        nc.sync.dma_start(out[:, r0 : r0 + BT], os_[:, :])

    fc1(0)
    for t in range(NBT):
        if t + 1 < NBT:
            fc1(t + 1)
        fc2(t)
        if t >= 1:
            fc3(t - 1)
    fc3(NBT - 1)


def _build_c(conv_w: np.ndarray) -> np.ndarray:
    """C[p, q] with conv(x).flat = x @ C. Pure scatter of conv_w."""
    c = np.zeros((NPIX, FLAT), np.float32)
    oi = np.arange(OHW)
    oj = np.arange(OHW)
    q = (oi[:, None] * OHW + oj[None, :]).ravel()
    for dy in range(3):
        for dx in range(3):
            p = ((oi[:, None] + dy) * IMG + (oj[None, :] + dx)).ravel()
            c[p, q] = conv_w[dy, dx]
    return c


_NC_CACHE: list = []


def _get_nc():
    if _NC_CACHE:
        return _NC_CACHE[0]
    nc = bass.Bass("TRN2", target_bir_lowering=False, debug=False)
    xt_d = nc.dram_tensor("xt", [NPIX, B_CORE], BF16, kind="ExternalInput").ap()
    wb = nc.dram_tensor("wb", [128, WCOLS], BF16, kind="ExternalInput").ap()
    bb = nc.dram_tensor("bb", [128, 7], F32, kind="ExternalInput").ap()
    out = nc.dram_tensor("out", [NCLS, B_CORE], F32, kind="ExternalOutput").ap()
    with tile.TileContext(nc) as tc:
        with ExitStack() as ctx:
            _emit(ctx, tc, xt_d, wb, bb, out)
    _legalize_single_wait(nc)
    _NC_CACHE.append(nc)
    return nc


def _in_maps(inputs: dict) -> list:
    x = np.asarray(inputs["x"], dtype=np.float32)
    assert x.shape == (B_FULL, NPIX), x.shape
    c = _build_c(np.asarray(inputs["conv_w"], dtype=np.float32))
    w1p = (c @ np.asarray(inputs["W1"], np.float32)).astype(NP_BF16)
    w2 = np.asarray(inputs["W2"], np.float32).astype(NP_BF16)
    w3 = np.asarray(inputs["W3"], np.float32).astype(NP_BF16)
    wb = np.zeros((128, WCOLS), NP_BF16)
    for pc in range(NPC):
        wb[0:PW, W1P_OFF + pc * HID : W1P_OFF + (pc + 1) * HID] = w1p[
            pc * PW : (pc + 1) * PW
        ]
    for hc, (h0, hp) in enumerate(H_CH):
        wb[0:hp, W2_OFF + hc * HID : W2_OFF + hc * HID + HID] = w2[h0 : h0 + hp]
        wb[0:hp, W3_OFF + hc * NCLS : W3_OFF + (hc + 1) * NCLS] = w3[h0 : h0 + hp]
    bbl = np.zeros((128, 7), np.float32)
    b1 = np.asarray(inputs["b1"], np.float32)
    b2 = np.asarray(inputs["b2"], np.float32)
    b3 = np.asarray(inputs["b3"], np.float32)
    for hc, (h0, hp) in enumerate(H_CH):
        bbl[0:hp, hc] = b1[h0 : h0 + hp]
        bbl[0:hp, 3 + hc] = b2[h0 : h0 + hp]
    bbl[0:NCLS, 6] = b3
    xb = x.astype(NP_BF16)
    common = {"wb": wb, "bb": bbl}
    return [
        {
            "xt": np.ascontiguousarray(xb[c_ * B_CORE : (c_ + 1) * B_CORE].T),
            **common,
        }
        for c_ in range(N_CORES)
    ]


def _gather(res) -> np.ndarray:
    return np.concatenate(
        [np.ascontiguousarray(res.results[c]["out"].T) for c in range(N_CORES)],
        axis=0,
    )


def kernel(**inputs) -> np.ndarray:
    nc = _get_nc()
    res = run_bass_kernel_spmd(nc, _in_maps(inputs), list(range(N_CORES)))
    return _gather(res)


if __name__ == "__main__":
    rng = np.random.default_rng(0)
    ins = {
        "x": rng.standard_normal((B_FULL, NPIX), dtype=np.float32),
        "conv_w": rng.standard_normal((3, 3), dtype=np.float32) * 0.1,
        "W1": rng.standard_normal((FLAT, HID), dtype=np.float32) * 0.04,
        "b1": np.zeros(HID, np.float32),
        "W2": rng.standard_normal((HID, HID), dtype=np.float32) * 0.06,
        "b2": np.zeros(HID, np.float32),
        "W3": rng.standard_normal((HID, NCLS), dtype=np.float32) * 0.06,
        "b3": np.zeros(NCLS, np.float32),
    }
    y = kernel(**ins)
    # numpy reference with explicit conv
    from numpy.lib.stride_tricks import sliding_window_view

    img = ins["x"].reshape(-1, IMG, IMG)
    win = sliding_window_view(img, (3, 3), axis=(1, 2))
    conv = np.einsum("bijkl,kl->bij", win, ins["conv_w"]).reshape(-1, FLAT)
    h = np.maximum(conv @ ins["W1"] + ins["b1"], 0)
    h = np.maximum(h @ ins["W2"] + ins["b2"], 0)
    ref = h @ ins["W3"] + ins["b3"]
    err = np.abs(y - ref).max() / (np.abs(ref).max() + 1e-9)
    print("max rel err vs numpy:", err)


# revision 16
# speedup vs baseline: 1.2762x; 1.0352x over previous
"""Trainium2 Bass kernel for DigitConvolutionalModel forward pass.

Model: x[B,784] -> 3x3 valid conv (single channel) -> flatten[676]
       -> relu(.@W1+b1) -> relu(.@W2+b2) -> .@W3+b3 -> [B,10]

Strategy:
  - Pure data parallel: batch 32768 sharded 8 ways (4096 rows/core);
    weights replicated.
  - The conv is linear, so it folds into the first Linear layer:
        conv(x).flat @ W1 == x @ (C @ W1)
    where C[784,676] is the conv-as-matrix. The host computes
    W1p = C @ W1 directly and pre-transposes x to pixel-major
    [784, 4096] per core, so the device DMAs matmul-ready [pix, batch]
    tiles and the PE spends zero cycles transposing inputs.
  - All matmul operands are bf16 (full-rate PE path); PSUM accumulation
    stays fp32, biases applied in fp32 on ScalarE during PSUM eviction.
    End-to-end bf16 error ~5e-3, well under the 2e-2 gate.
  - Pixel dim is chunked 7x112 so each batch-tile group loads with ONE
    large DMA; all weights+biases are packed into two [128, *] blobs
    loaded with one DMA each (per-DMA descriptor generation dominated
    the v3 startup). Input group DMAs alternate between the two HWDGE
    queues (sync + scalar engines); everything is pre-issued.
  - Output is stored TRANSPOSED [10, 4096] (host transposes back):
    the ps3 eviction lands batch-in-free-dim, so stores are 10
    contiguous 2KB descriptors instead of 512 40-byte ones, and no
    PE transpose / DVE cast is needed on the output path.
  - Last two tiles split into 256-col halves to pipeline the drain.
"""

import sys

for _p in (
    "/opt/trn_rl_repo",
    "/root/.axon_site",
    "/root/.axon_site/_ro/trn_rl_repo",
    "/root/.axon_site/_ro/pypackages",
):
    if _p not in sys.path:
        sys.path.append(_p)

from contextlib import ExitStack

import numpy as np
import ml_dtypes

import concourse.bass as bass
import concourse.tile as tile
from concourse import mybir
from concourse.bass_utils import run_bass_kernel_spmd
from concourse.masks import make_identity

F32 = mybir.dt.float32
BF16 = mybir.dt.bfloat16
AFT = mybir.ActivationFunctionType
NP_BF16 = ml_dtypes.bfloat16

B_FULL = 32768
N_CORES = 8
B_CORE = B_FULL // N_CORES  # 4096
IMG = 28
OHW = 26
FLAT = OHW * OHW  # 676
NPIX = IMG * IMG  # 784
HID = 300
NCLS = 10

BT = 512  # batch tile (matmul moving free dim; PSUM bank = 512 fp32)
NBT = B_CORE // BT  # 8

PW = 112  # pixel chunk: 784 = 7*112, uniform -> single-DMA group loads
NPC = NPIX // PW  # 7
H_CH = [(s, min(128, HID - s)) for s in range(0, HID, 128)]  # 3 chunks

# weight blobs: w1p gates fc1 startup, w2/w3 only gate fc2 ~5us later
WACOLS = NPC * HID  # 2100: 7 chunks of [112, 300]
WBCOLS = 3 * HID + 3 * NCLS  # 930: w2 3x[hp,300] then w3 3x[hp,10]
W3_OFF = 3 * HID

# batch-tile groups: [1, 2, 2, 2, 1] tiles; each group loads as TWO DMAs
# (pixel chunks 0-4 on the sync HWDGE queue, 5-6 on the scalar one) so
# the two queues' ~135GB/s each both contribute to the gating transfers
GROUPS = [(0, 1), (1, 2), (3, 2), (5, 2), (7, 1)]
ACH = 5  # chunks in the "a" half


def _legalize_single_wait(nc):
    """This walrus build accepts only one sync-wait per instruction; move
    extra waits onto NoOps inserted just before, on the same engine."""
    n = 0
    for fn in nc.m.functions:
        for bb in fn.blocks:
            new_insts = []
            for inst in bb.instructions:
                si = inst.sync_info
                if si is not None and si.on_wait and len(si.on_wait) > 1:
                    waits = list(si.on_wait)
                    for w in waits[:-1]:
                        nop = mybir.InstNoOp(
                            name=f"{inst.name}-w{n}",
                            sync_info=mybir.SyncInfo(on_wait=[w], on_update=[]),
                            bass_nofuse=True,
                            engine=inst.engine,
                        )
                        n += 1
                        nc.register_instruction(nop, overwrite=True)
                        new_insts.append(nop)
                    inst.sync_info = mybir.SyncInfo(
                        on_wait=[waits[-1]], on_update=list(si.on_update)
                    )
                new_insts.append(inst)
            bb.instructions = new_insts
    return n


def _emit(ctx: ExitStack, tc: tile.TileContext, xt_d, wbA, wbB, bb, out):
    nc = tc.nc

    const = ctx.enter_context(tc.tile_pool(name="const", bufs=1))
    psmm = ctx.enter_context(tc.tile_pool(name="psmm", bufs=8, space="PSUM"))
    hp_ = ctx.enter_context(tc.tile_pool(name="hp", bufs=2))
    obp = ctx.enter_context(tc.tile_pool(name="obp", bufs=8))

    ident = const.tile([128, 128], F32, name="ident")
    make_identity(nc, ident)
    identb = const.tile([128, 128], BF16, name="identb")
    nc.vector.tensor_copy(identb[:, :], ident[:, :])

    # --- all DMAs pre-issued, gating transfers split across both HWDGE
    # queues: sync carries x chunks 0-4 (+ biases + w2/w3), scalar
    # carries w1p + x chunks 5-6 ---
    xt_v = xt_d.rearrange("(c p) b -> p c b", c=NPC)
    ga0 = const.tile([PW, ACH, BT], BF16, name="xg0a")
    nc.sync.dma_start(ga0[:, :, :], xt_v[:, 0:ACH, 0:BT])
    bbt = const.tile([128, 7], F32, name="bbt")
    nc.sync.dma_start(bbt[:, :], bb[:, :])
    wbbt = const.tile([128, WBCOLS], BF16, name="wbbt")
    nc.sync.dma_start(wbbt[:, :], wbB[:, :])
    wbat = const.tile([PW, NPC, HID], BF16, name="wbat")
    nc.scalar.dma_start(wbat[:, :, :], wbA.rearrange("(c p) h -> p c h", c=NPC))
    gb0 = const.tile([PW, NPC - ACH, BT], BF16, name="xg0b")
    nc.scalar.dma_start(gb0[:, :, :], xt_v[:, ACH:NPC, 0:BT])
    xga, xgb = [ga0], [gb0]
    for gi, (t0, nt) in enumerate(GROUPS[1:], 1):
        ga = const.tile([PW, ACH, nt * BT], BF16, name=f"xg{gi}a")
        nc.sync.dma_start(ga[:, :, :], xt_v[:, 0:ACH, t0 * BT : (t0 + nt) * BT])
        gb = const.tile([PW, NPC - ACH, nt * BT], BF16, name=f"xg{gi}b")
        nc.scalar.dma_start(gb[:, :, :], xt_v[:, ACH:NPC, t0 * BT : (t0 + nt) * BT])
        xga.append(ga)
        xgb.append(gb)

    w1ps = [wbat[:, pc, :] for pc in range(NPC)]
    w2s = [wbbt[0:hp, hc * HID : hc * HID + HID] for hc, (h0, hp) in enumerate(H_CH)]
    w3s = [wbbt[0:hp, W3_OFF + hc * NCLS : W3_OFF + (hc + 1) * NCLS] for hc, (h0, hp) in enumerate(H_CH)]
    b1s = [bbt[0:hp, hc : hc + 1] for hc, (h0, hp) in enumerate(H_CH)]
    b2s = [bbt[0:hp, 3 + hc : 4 + hc] for hc, (h0, hp) in enumerate(H_CH)]
    b3s = bbt[0:NCLS, 6:7]

    # Dense PE warmup burst: keeps the HAM clock gate ramping while the
    # gating DMAs (~5us/queue) land; N=512 so rep timing is predictable
    dummy = const.tile([128, BT], BF16, name="dummy")
    nc.gpsimd.memset(dummy[:, :], 0.0)
    warm = psmm.tile([128, BT], F32, name="warm", tag="psf")
    for _ in range(32):
        nc.tensor.matmul(
            warm[0:128, 0:BT], identb[:, :], dummy[:, :],
            start=True, stop=True,
        )

    # tile index -> (group index, col offset of tile in group)
    tview = {}
    for gi, (t0, nt) in enumerate(GROUPS):
        for k in range(nt):
            tview[t0 + k] = (gi, k * BT)

    def xchunk(t, pc):
        gi, goff = tview[t]
        if pc < ACH:
            return xga[gi][:, pc, goff : goff + BT]
        return xgb[gi][:, pc - ACH, goff : goff + BT]

    # --- software-pipelined main loop: iteration I_t issues
    #     fc1(t+1) | fc2(t) | fc3(t-1)
    # so the PE never stalls at an fc boundary waiting for a ScalarE
    # PSUM eviction — the evictions of stage s(t) overlap the ~4.5us of
    # fc1(t+1) matmuls. PSUM live set: 3 (ps1) + 3 (ps2) + 1 (ps3) = 7
    # of 8 banks. All matmul runs are same-PSUM-bank chains (hc-outer /
    # m-outer): consecutive bank-switching accumulation was measured
    # ~1.2us/tile slower.
    h1v, h2v = {}, {}

    def fc1(t):
        h1 = []
        for hc, (h0, hp) in enumerate(H_CH):
            ps = psmm.tile([128, BT], F32, name=f"ps1_{hc}", tag="psf")
            for pc in range(NPC):
                nc.tensor.matmul(
                    ps[0:hp, 0:BT],
                    w1ps[pc][0:PW, h0 : h0 + hp],
                    xchunk(t, pc),
                    start=(pc == 0),
                    stop=(pc == NPC - 1),
                )
            h = hp_.tile([hp, BT], BF16, name=f"h1_{hc}", tag=f"h1_{hc}")
            nc.scalar.activation(h[:, :], ps[0:hp, 0:BT], AFT.Relu, bias=b1s[hc])
            h1.append(h)
        h1v[t] = h1

    def fc2(t):
        h1 = h1v.pop(t)
        h2 = []
        for hc2, (g0, gp) in enumerate(H_CH):
            ps = psmm.tile([128, BT], F32, name=f"ps2_{hc2}", tag="psf")
            for hc, (h0, hp) in enumerate(H_CH):
                nc.tensor.matmul(
                    ps[0:gp, 0:BT],
                    w2s[hc][0:hp, g0 : g0 + gp],
                    h1[hc][0:hp, 0:BT],
                    start=(hc == 0),
                    stop=(hc == len(H_CH) - 1),
                )
            h = hp_.tile([gp, BT], BF16, name=f"h2_{hc2}", tag=f"h2_{hc2}")
            nc.scalar.activation(h[:, :], ps[0:gp, 0:BT], AFT.Relu, bias=b2s[hc2])
            h2.append(h)
        h2v[t] = h2

    def fc3(t):
        h2 = h2v.pop(t)
        ps = psmm.tile([128, BT], F32, name="ps3", tag="psf")
        for hc, (h0, hp) in enumerate(H_CH):
            nc.tensor.matmul(
                ps[0:NCLS, 0:BT],
                w3s[hc][0:hp, 0:NCLS],
                h2[hc][0:hp, 0:BT],
                start=(hc == 0),
                stop=(hc == len(H_CH) - 1),
            )
        os_ = obp.tile([NCLS, BT], F32, name="os", tag="os")
        nc.scalar.activation(os_[:, :], ps[0:NCLS, 0:BT], AFT.Identity, bias=b3s)
        r0 = t * BT
        # mid-loop outs ride the idle gpsimd software-DGE queue; the final
        # one stays on the fast sync HWDGE so the epilogue isn't left
        # waiting on a software-generated descriptor chain
        eng = nc.sync if t == NBT - 1 else nc.gpsimd
        eng.dma_start(out[:, r0 : r0 + BT], os_[:, :])

    fc1(0)
    for t in range(NBT):
        if t + 1 < NBT:
            fc1(t + 1)
        fc2(t)
        if t >= 1:
            fc3(t - 1)
    fc3(NBT - 1)


def _build_c(conv_w: np.ndarray) -> np.ndarray:
    """C[p, q] with conv(x).flat = x @ C. Pure scatter of conv_w."""
    c = np.zeros((NPIX, FLAT), np.float32)
    oi = np.arange(OHW)
    oj = np.arange(OHW)
    q = (oi[:, None] * OHW + oj[None, :]).ravel()
    for dy in range(3):
        for dx in range(3):
            p = ((oi[:, None] + dy) * IMG + (oj[None, :] + dx)).ravel()
            c[p, q] = conv_w[dy, dx]
    return c


_NC_CACHE: list = []


def _get_nc():
    if _NC_CACHE:
        return _NC_CACHE[0]
    nc = bass.Bass("TRN2", target_bir_lowering=False, debug=False)
    xt_d = nc.dram_tensor("xt", [NPIX, B_CORE], BF16, kind="ExternalInput").ap()
    wbA = nc.dram_tensor("wba", [NPIX, HID], BF16, kind="ExternalInput").ap()
    wbB = nc.dram_tensor("wbb", [128, WBCOLS], BF16, kind="ExternalInput").ap()
    bb = nc.dram_tensor("bb", [128, 7], F32, kind="ExternalInput").ap()
    out = nc.dram_tensor("out", [NCLS, B_CORE], F32, kind="ExternalOutput").ap()
    with tile.TileContext(nc) as tc:
        with ExitStack() as ctx:
            _emit(ctx, tc, xt_d, wbA, wbB, bb, out)
    _legalize_single_wait(nc)
    _NC_CACHE.append(nc)
    return nc


def _in_maps(inputs: dict) -> list:
    x = np.asarray(inputs["x"], dtype=np.float32)
    assert x.shape == (B_FULL, NPIX), x.shape
    c = _build_c(np.asarray(inputs["conv_w"], dtype=np.float32))
    w1p = np.ascontiguousarray(
        (c @ np.asarray(inputs["W1"], np.float32)).astype(NP_BF16)
    )
    w2 = np.asarray(inputs["W2"], np.float32).astype(NP_BF16)
    w3 = np.asarray(inputs["W3"], np.float32).astype(NP_BF16)
    wbb = np.zeros((128, WBCOLS), NP_BF16)
    for hc, (h0, hp) in enumerate(H_CH):
        wbb[0:hp, hc * HID : hc * HID + HID] = w2[h0 : h0 + hp]
        wbb[0:hp, W3_OFF + hc * NCLS : W3_OFF + (hc + 1) * NCLS] = w3[h0 : h0 + hp]
    bbl = np.zeros((128, 7), np.float32)
    b1 = np.asarray(inputs["b1"], np.float32)
    b2 = np.asarray(inputs["b2"], np.float32)
    b3 = np.asarray(inputs["b3"], np.float32)
    for hc, (h0, hp) in enumerate(H_CH):
        bbl[0:hp, hc] = b1[h0 : h0 + hp]
        bbl[0:hp, 3 + hc] = b2[h0 : h0 + hp]
    bbl[0:NCLS, 6] = b3
    xb = x.astype(NP_BF16)
    common = {"wba": w1p, "wbb": wbb, "bb": bbl}
    return [
        {
            "xt": np.ascontiguousarray(xb[c_ * B_CORE : (c_ + 1) * B_CORE].T),
            **common,
        }
        for c_ in range(N_CORES)
    ]


def _gather(res) -> np.ndarray:
    return np.concatenate(
        [np.ascontiguousarray(res.results[c]["out"].T) for c in range(N_CORES)],
        axis=0,
    )


def kernel(**inputs) -> np.ndarray:
    nc = _get_nc()
    res = run_bass_kernel_spmd(nc, _in_maps(inputs), list(range(N_CORES)))
    return _gather(res)


if __name__ == "__main__":
    rng = np.random.default_rng(0)
    ins = {
        "x": rng.standard_normal((B_FULL, NPIX), dtype=np.float32),
        "conv_w": rng.standard_normal((3, 3), dtype=np.float32) * 0.1,
        "W1": rng.standard_normal((FLAT, HID), dtype=np.float32) * 0.04,
        "b1": np.zeros(HID, np.float32),
        "W2": rng.standard_normal((HID, HID), dtype=np.float32) * 0.06,
        "b2": np.zeros(HID, np.float32),
        "W3": rng.standard_normal((HID, NCLS), dtype=np.float32) * 0.06,
        "b3": np.zeros(NCLS, np.float32),
    }
    y = kernel(**ins)
    # numpy reference with explicit conv
    from numpy.lib.stride_tricks import sliding_window_view

    img = ins["x"].reshape(-1, IMG, IMG)
    win = sliding_window_view(img, (3, 3), axis=(1, 2))
    conv = np.einsum("bijkl,kl->bij", win, ins["conv_w"]).reshape(-1, FLAT)
    h = np.maximum(conv @ ins["W1"] + ins["b1"], 0)
    h = np.maximum(h @ ins["W2"] + ins["b2"], 0)
    ref = h @ ins["W3"] + ins["b3"]
    err = np.abs(y - ref).max() / (np.abs(ref).max() + 1e-9)
    print("max rel err vs numpy:", err)


# revision 19
# speedup vs baseline: 1.3741x; 1.0767x over previous
"""Trainium2 Bass kernel for DigitConvolutionalModel forward pass.

Model: x[B,784] -> 3x3 valid conv (single channel) -> flatten[676]
       -> relu(.@W1+b1) -> relu(.@W2+b2) -> .@W3+b3 -> [B,10]

Strategy:
  - Pure data parallel: batch 32768 sharded 8 ways (4096 rows/core);
    weights replicated.
  - The conv is linear, so it folds into the first Linear layer:
        conv(x).flat @ W1 == x @ (C @ W1)
    where C[784,676] is the conv-as-matrix. The host computes
    W1p = C @ W1 directly and pre-transposes x to pixel-major
    [784, 4096] per core, so the device DMAs matmul-ready [pix, batch]
    tiles and the PE spends zero cycles transposing inputs.
  - All matmul operands are bf16 (full-rate PE path); PSUM accumulation
    stays fp32, biases applied in fp32 on ScalarE during PSUM eviction.
    End-to-end bf16 error ~5e-3, well under the 2e-2 gate.
  - Pixel dim is chunked 7x112 so each batch-tile group loads with ONE
    large DMA; all weights+biases are packed into two [128, *] blobs
    loaded with one DMA each (per-DMA descriptor generation dominated
    the v3 startup). Input group DMAs alternate between the two HWDGE
    queues (sync + scalar engines); everything is pre-issued.
  - Output is stored TRANSPOSED [10, 4096] (host transposes back):
    the ps3 eviction lands batch-in-free-dim, so stores are 10
    contiguous 2KB descriptors instead of 512 40-byte ones, and no
    PE transpose / DVE cast is needed on the output path.
  - Last two tiles split into 256-col halves to pipeline the drain.
"""

import sys

for _p in (
    "/opt/trn_rl_repo",
    "/root/.axon_site",
    "/root/.axon_site/_ro/trn_rl_repo",
    "/root/.axon_site/_ro/pypackages",
):
    if _p not in sys.path:
        sys.path.append(_p)

from contextlib import ExitStack

import numpy as np
import ml_dtypes

import concourse.bass as bass
import concourse.tile as tile
from concourse import mybir
from concourse.bass_utils import run_bass_kernel_spmd
from concourse.masks import make_identity

F32 = mybir.dt.float32
BF16 = mybir.dt.bfloat16
AFT = mybir.ActivationFunctionType
NP_BF16 = ml_dtypes.bfloat16

B_FULL = 32768
N_CORES = 8
B_CORE = B_FULL // N_CORES  # 4096
IMG = 28
OHW = 26
FLAT = OHW * OHW  # 676
NPIX = IMG * IMG  # 784
HID = 300
NCLS = 10

BT = 512  # batch tile (matmul moving free dim; PSUM bank = 512 fp32)
NBT = B_CORE // BT  # 8

PW = 112  # pixel chunk: 784 = 7*112, uniform -> single-DMA group loads
NPC = NPIX // PW  # 7
H_CH = [(s, min(128, HID - s)) for s in range(0, HID, 128)]  # 3 chunks

# weight blobs: w1p gates fc1 startup, w2/w3 only gate fc2 ~5us later
WACOLS = NPC * HID  # 2100: 7 chunks of [112, 300]
WBCOLS = 3 * HID + 3 * NCLS  # 930: w2 3x[hp,300] then w3 3x[hp,10]
W3_OFF = 3 * HID

# batch-tile groups: [1, 2, 2, 2, 1] tiles; each group loads as TWO DMAs
# (pixel chunks 0-4 on the sync HWDGE queue, 5-6 on the scalar one) so
# the two queues' ~135GB/s each both contribute to the gating transfers
GROUPS = [(0, 1), (1, 1), (2, 2), (4, 2), (6, 2)]
ACH = 5  # chunks in the "a" half


def _legalize_single_wait(nc):
    """This walrus build accepts only one sync-wait per instruction; move
    extra waits onto NoOps inserted just before, on the same engine."""
    n = 0
    for fn in nc.m.functions:
        for bb in fn.blocks:
            new_insts = []
            for inst in bb.instructions:
                si = inst.sync_info
                if si is not None and si.on_wait and len(si.on_wait) > 1:
                    waits = list(si.on_wait)
                    for w in waits[:-1]:
                        nop = mybir.InstNoOp(
                            name=f"{inst.name}-w{n}",
                            sync_info=mybir.SyncInfo(on_wait=[w], on_update=[]),
                            bass_nofuse=True,
                            engine=inst.engine,
                        )
                        n += 1
                        nc.register_instruction(nop, overwrite=True)
                        new_insts.append(nop)
                    inst.sync_info = mybir.SyncInfo(
                        on_wait=[waits[-1]], on_update=list(si.on_update)
                    )
                new_insts.append(inst)
            bb.instructions = new_insts
    return n


def _emit(ctx: ExitStack, tc: tile.TileContext, xt_d, wbA, wbB, bb, out):
    nc = tc.nc

    const = ctx.enter_context(tc.tile_pool(name="const", bufs=1))
    psmm = ctx.enter_context(tc.tile_pool(name="psmm", bufs=8, space="PSUM"))
    hp_ = ctx.enter_context(tc.tile_pool(name="hp", bufs=2))
    obp = ctx.enter_context(tc.tile_pool(name="obp", bufs=8))

    # zeroed dummy for the PE warmup burst (memset first: nothing ahead of
    # it on gpsimd, so the warmup can start right after the preamble)
    dummy = const.tile([128, BT], BF16, name="dummy")
    nc.gpsimd.memset(dummy[:, :], 0.0)

    # --- all DMAs pre-issued, gating transfers split across both HWDGE
    # queues: sync carries x chunks 0-4 (+ biases + w2/w3), scalar
    # carries w1p + x chunks 5-6 ---
    xt_v = xt_d.rearrange("(c p) b -> p c b", c=NPC)
    ga0 = const.tile([PW, ACH, BT], BF16, name="xg0a")
    nc.sync.dma_start(ga0[:, :, :], xt_v[:, 0:ACH, 0:BT])
    bbt = const.tile([128, 7], F32, name="bbt")
    nc.sync.dma_start(bbt[:, :], bb[:, :])
    wbbt = const.tile([128, WBCOLS], BF16, name="wbbt")
    nc.sync.dma_start(wbbt[:, :], wbB[:, :])
    wbat = const.tile([PW, NPC, HID], BF16, name="wbat")
    nc.scalar.dma_start(wbat[:, :, :], wbA.rearrange("(c p) h -> p c h", c=NPC))
    gb0 = const.tile([PW, NPC - ACH, BT], BF16, name="xg0b")
    nc.scalar.dma_start(gb0[:, :, :], xt_v[:, ACH:NPC, 0:BT])
    xga, xgb = [ga0], [gb0]
    for gi, (t0, nt) in enumerate(GROUPS[1:], 1):
        ga = const.tile([PW, ACH, nt * BT], BF16, name=f"xg{gi}a")
        nc.sync.dma_start(ga[:, :, :], xt_v[:, 0:ACH, t0 * BT : (t0 + nt) * BT])
        gb = const.tile([PW, NPC - ACH, nt * BT], BF16, name=f"xg{gi}b")
        nc.scalar.dma_start(gb[:, :, :], xt_v[:, ACH:NPC, t0 * BT : (t0 + nt) * BT])
        xga.append(ga)
        xgb.append(gb)

    w1ps = [wbat[:, pc, :] for pc in range(NPC)]
    w2s = [wbbt[0:hp, hc * HID : hc * HID + HID] for hc, (h0, hp) in enumerate(H_CH)]
    w3s = [wbbt[0:hp, W3_OFF + hc * NCLS : W3_OFF + (hc + 1) * NCLS] for hc, (h0, hp) in enumerate(H_CH)]
    b1s = [bbt[0:hp, hc : hc + 1] for hc, (h0, hp) in enumerate(H_CH)]
    b2s = [bbt[0:hp, 3 + hc : 4 + hc] for hc, (h0, hp) in enumerate(H_CH)]
    b3s = bbt[0:NCLS, 6:7]

    # Dense PE warmup burst: keeps the HAM clock gate ramping while the
    # gating DMAs (~5us/queue) land; N=512 so rep timing is predictable
    warm = psmm.tile([128, BT], F32, name="warm", tag="psf")
    for _ in range(16):
        nc.tensor.matmul(
            warm[0:128, 0:BT], dummy[:, 0:128], dummy[:, :],
            start=True, stop=True,
        )

    # tile index -> (group index, col offset of tile in group)
    tview = {}
    for gi, (t0, nt) in enumerate(GROUPS):
        for k in range(nt):
            tview[t0 + k] = (gi, k * BT)

    def xchunk(t, pc):
        gi, goff = tview[t]
        if pc < ACH:
            return xga[gi][:, pc, goff : goff + BT]
        return xgb[gi][:, pc - ACH, goff : goff + BT]

    # --- software-pipelined main loop: iteration I_t issues
    #     fc1(t+1) | fc2(t) | fc3(t-1)
    # so the PE never stalls at an fc boundary waiting for a ScalarE
    # PSUM eviction — the evictions of stage s(t) overlap the ~4.5us of
    # fc1(t+1) matmuls. PSUM live set: 3 (ps1) + 3 (ps2) + 1 (ps3) = 7
    # of 8 banks. All matmul runs are same-PSUM-bank chains (hc-outer /
    # m-outer): consecutive bank-switching accumulation was measured
    # ~1.2us/tile slower.
    h1v, h2v = {}, {}

    def fc1(t):
        h1 = []
        for hc, (h0, hp) in enumerate(H_CH):
            ps = psmm.tile([128, BT], F32, name=f"ps1_{hc}", tag="psf")
            for pc in range(NPC):
                nc.tensor.matmul(
                    ps[0:hp, 0:BT],
                    w1ps[pc][0:PW, h0 : h0 + hp],
                    xchunk(t, pc),
                    start=(pc == 0),
                    stop=(pc == NPC - 1),
                )
            h = hp_.tile([hp, BT], BF16, name=f"h1_{hc}", tag=f"h1_{hc}")
            nc.scalar.activation(h[:, :], ps[0:hp, 0:BT], AFT.Relu, bias=b1s[hc])
            h1.append(h)
        h1v[t] = h1

    def fc2(t):
        h1 = h1v.pop(t)
        h2 = []
        for hc2, (g0, gp) in enumerate(H_CH):
            ps = psmm.tile([128, BT], F32, name=f"ps2_{hc2}", tag="psf")
            for hc, (h0, hp) in enumerate(H_CH):
                nc.tensor.matmul(
                    ps[0:gp, 0:BT],
                    w2s[hc][0:hp, g0 : g0 + gp],
                    h1[hc][0:hp, 0:BT],
                    start=(hc == 0),
                    stop=(hc == len(H_CH) - 1),
                )
            h = hp_.tile([gp, BT], BF16, name=f"h2_{hc2}", tag=f"h2_{hc2}")
            nc.scalar.activation(h[:, :], ps[0:gp, 0:BT], AFT.Relu, bias=b2s[hc2])
            h2.append(h)
        h2v[t] = h2

    def fc3(t):
        h2 = h2v.pop(t)
        ps = psmm.tile([128, BT], F32, name="ps3", tag="psf")
        for hc, (h0, hp) in enumerate(H_CH):
            nc.tensor.matmul(
                ps[0:NCLS, 0:BT],
                w3s[hc][0:hp, 0:NCLS],
                h2[hc][0:hp, 0:BT],
                start=(hc == 0),
                stop=(hc == len(H_CH) - 1),
            )
        os_ = obp.tile([NCLS, BT], F32, name="os", tag="os")
        nc.scalar.activation(os_[:, :], ps[0:NCLS, 0:BT], AFT.Identity, bias=b3s)
        r0 = t * BT
        # mid-loop outs ride the idle gpsimd software-DGE queue; the final
        # one stays on the fast sync HWDGE so the epilogue isn't left
        # waiting on a software-generated descriptor chain
        eng = nc.sync if t == NBT - 1 else nc.gpsimd
        eng.dma_start(out[:, r0 : r0 + BT], os_[:, :])

    fc1(0)
    for t in range(NBT):
        if t + 1 < NBT:
            fc1(t + 1)
        fc2(t)
        if t >= 1:
            fc3(t - 1)
    fc3(NBT - 1)


def _build_c(conv_w: np.ndarray) -> np.ndarray:
    """C[p, q] with conv(x).flat = x @ C. Pure scatter of conv_w."""
    c = np.zeros((NPIX, FLAT), np.float32)
    oi = np.arange(OHW)
    oj = np.arange(OHW)
    q = (oi[:, None] * OHW + oj[None, :]).ravel()
    for dy in range(3):
        for dx in range(3):
            p = ((oi[:, None] + dy) * IMG + (oj[None, :] + dx)).ravel()
            c[p, q] = conv_w[dy, dx]
    return c


_NC_CACHE: list = []


def _get_nc():
    if _NC_CACHE:
        return _NC_CACHE[0]
    nc = bass.Bass("TRN2", target_bir_lowering=False, debug=False)
    xt_d = nc.dram_tensor("xt", [NPIX, B_CORE], BF16, kind="ExternalInput").ap()
    wbA = nc.dram_tensor("wba", [NPIX, HID], BF16, kind="ExternalInput").ap()
    wbB = nc.dram_tensor("wbb", [128, WBCOLS], BF16, kind="ExternalInput").ap()
    bb = nc.dram_tensor("bb", [128, 7], F32, kind="ExternalInput").ap()
    out = nc.dram_tensor("out", [NCLS, B_CORE], F32, kind="ExternalOutput").ap()
    with tile.TileContext(nc) as tc:
        with ExitStack() as ctx:
            _emit(ctx, tc, xt_d, wbA, wbB, bb, out)
    _legalize_single_wait(nc)
    _NC_CACHE.append(nc)
    return nc


def _in_maps(inputs: dict) -> list:
    x = np.asarray(inputs["x"], dtype=np.float32)
    assert x.shape == (B_FULL, NPIX), x.shape
    c = _build_c(np.asarray(inputs["conv_w"], dtype=np.float32))
    w1p = np.ascontiguousarray(
        (c @ np.asarray(inputs["W1"], np.float32)).astype(NP_BF16)
    )
    w2 = np.asarray(inputs["W2"], np.float32).astype(NP_BF16)
    w3 = np.asarray(inputs["W3"], np.float32).astype(NP_BF16)
    wbb = np.zeros((128, WBCOLS), NP_BF16)
    for hc, (h0, hp) in enumerate(H_CH):
        wbb[0:hp, hc * HID : hc * HID + HID] = w2[h0 : h0 + hp]
        wbb[0:hp, W3_OFF + hc * NCLS : W3_OFF + (hc + 1) * NCLS] = w3[h0 : h0 + hp]
    bbl = np.zeros((128, 7), np.float32)
    b1 = np.asarray(inputs["b1"], np.float32)
    b2 = np.asarray(inputs["b2"], np.float32)
    b3 = np.asarray(inputs["b3"], np.float32)
    for hc, (h0, hp) in enumerate(H_CH):
        bbl[0:hp, hc] = b1[h0 : h0 + hp]
        bbl[0:hp, 3 + hc] = b2[h0 : h0 + hp]
    bbl[0:NCLS, 6] = b3
    xb = x.astype(NP_BF16)
    common = {"wba": w1p, "wbb": wbb, "bb": bbl}
    return [
        {
            "xt": np.ascontiguousarray(xb[c_ * B_CORE : (c_ + 1) * B_CORE].T),
            **common,
        }
        for c_ in range(N_CORES)
    ]


def _gather(res) -> np.ndarray:
    return np.concatenate(
        [np.ascontiguousarray(res.results[c]["out"].T) for c in range(N_CORES)],
        axis=0,
    )


def kernel(**inputs) -> np.ndarray:
    nc = _get_nc()
    res = run_bass_kernel_spmd(nc, _in_maps(inputs), list(range(N_CORES)))
    return _gather(res)


if __name__ == "__main__":
    rng = np.random.default_rng(0)
    ins = {
        "x": rng.standard_normal((B_FULL, NPIX), dtype=np.float32),
        "conv_w": rng.standard_normal((3, 3), dtype=np.float32) * 0.1,
        "W1": rng.standard_normal((FLAT, HID), dtype=np.float32) * 0.04,
        "b1": np.zeros(HID, np.float32),
        "W2": rng.standard_normal((HID, HID), dtype=np.float32) * 0.06,
        "b2": np.zeros(HID, np.float32),
        "W3": rng.standard_normal((HID, NCLS), dtype=np.float32) * 0.06,
        "b3": np.zeros(NCLS, np.float32),
    }
    y = kernel(**ins)
    # numpy reference with explicit conv
    from numpy.lib.stride_tricks import sliding_window_view

    img = ins["x"].reshape(-1, IMG, IMG)
    win = sliding_window_view(img, (3, 3), axis=(1, 2))
    conv = np.einsum("bijkl,kl->bij", win, ins["conv_w"]).reshape(-1, FLAT)
    h = np.maximum(conv @ ins["W1"] + ins["b1"], 0)
    h = np.maximum(h @ ins["W2"] + ins["b2"], 0)
    ref = h @ ins["W3"] + ins["b3"]
    err = np.abs(y - ref).max() / (np.abs(ref).max() + 1e-9)
    print("max rel err vs numpy:", err)
